# revision 1
# baseline (speedup 1.0000x reference)
"""Multi-head attention block (b=4, n=2048, d=256, h=8) on 8 TRN2 NeuronCores.

Sharding: core c handles (batch bi=c//2, query-half qh=c%2): it computes
K/V for the full sequence of its batch and Q for its 1024-row query half,
producing 1024 complete rows of the final output (host concatenates and
adds b_out; no cross-core reduction).

Design (all matmul operands fp16; PSUM fp32):
  - kT_c[hg][c] [128,512]: 4 heads' K^T stacked per seq chunk (per-chunk
    tiles so the woven emission cannot create tile-granularity dep cycles).
  - qT_pad[h] [128,1024]: per-head Q^T zero-padded to 128 partitions, so the
    scores matmul runs K=128 (the zero rows of qT_pad mask the other heads'
    rows of kT_c). Single tile-position, full-array matmuls only —
    tile_position packing crashes this device.
  - q-chunks of 256: one scores psum tensor [128,4,256] (2 banks) holds all
    4 heads of a head-group for one k-tile; ONE exp [128,1024] per k-tile.
  - AV: [v|ones] lhsT [128,33] folds the softmax denominator (row 32) into
    output row 32; one av accumulator [33,4,256] (2 banks; start=True only
    on each bank's first slice since it clears has_written bank-wide).
    PSUM: 4 (scores) + 2 (av) + 2 (projections/outproj) = 8 banks.
  - QKV/V projection units are woven into the attention emission stream as
    PE filler to keep the tensor engine dense (clock stays ~2 GHz).
  - Normalization: denominator rows -> DRAM bounce -> batched reciprocal
    [128,8] per q-chunk -> broadcast-read -> DVE multiply -> outT (fp16);
    output projection emitted per q-chunk as soon as outT is ready.
"""
import numpy as np

import concourse.bacc as bacc
import concourse.bass as bass
import concourse.mybir as mybir
import concourse.tile as tile
from concourse.bass_utils import run_bass_kernel_spmd

F32 = mybir.dt.float32
F16 = mybir.dt.float16
Exp = mybir.ActivationFunctionType.Exp
Copy = mybir.ActivationFunctionType.Copy

B, N, D = 4, 2048, 256
H, DH = 8, 32
NQ = N // 2            # per-core query rows
SCALE = D ** -0.5      # 0.0625
NKT = N // 128         # 16 k-tiles
QC = 256               # q-chunk
NQC = NQ // QC         # 4 q-chunks per core

_BUILD_CACHE = {}


def build():
    if "nc" in _BUILD_CACHE:
        return _BUILD_CACHE["nc"]
    nc = bacc.Bacc()

    xT_d = nc.dram_tensor("xT", [D, N], F32, kind="ExternalInput")
    xqT_d = nc.dram_tensor("xqT", [D, NQ], F32, kind="ExternalInput")
    w_d = nc.dram_tensor("w_qkv", [D, 3 * D], F32, kind="ExternalInput")
    b_d = nc.dram_tensor("b_qkv", [1, 3 * D], F32, kind="ExternalInput")
    wo_d = nc.dram_tensor("w_out", [D, D], F32, kind="ExternalInput")
    out_d = nc.dram_tensor("out", [NQ, D], F32, kind="ExternalOutput")
    den_dram = nc.dram_tensor("den_scratch", [2, NQC, 2, 512], F32)
    recip_dram = nc.dram_tensor("recip_scratch", [2, NQC, 2, 512], F32)

    with tile.TileContext(nc) as tc:
        with (
            tc.tile_pool(name="persist", bufs=1) as persist,
            tc.tile_pool(name="probs", bufs=4) as prpool,
            tc.tile_pool(name="avsb", bufs=3) as avsb_pool,
            tc.tile_pool(name="norm", bufs=4) as norm_pool,
            tc.tile_pool(name="outsb", bufs=3) as out_pool,
            tc.tile_pool(name="kqps", bufs=2, space="PSUM") as kqps,
            tc.tile_pool(name="scps", bufs=2, space="PSUM") as scps,
            tc.tile_pool(name="avps", bufs=1, space="PSUM") as avps,
        ):
            # ---- persistent tiles / loads ----
            ones = persist.tile([1, 512], F16, name="ones")
            nc.vector.memset(ones, 1.0)

            w_sb = [persist.tile([128, 3 * D], F16, name=f"w{d2}") for d2 in range(2)]
            b_sb = persist.tile([1, 3 * D], F16, name="b_sb")
            xT_sb = [persist.tile([128, N], F16, name=f"xT{d2}") for d2 in range(2)]
            xqT_sb = [persist.tile([128, NQ], F16, name=f"xq{d2}") for d2 in range(2)]
            wo_sb = [persist.tile([128, D], F16, name=f"wo{g}") for g in range(2)]
            for d2 in range(2):
                nc.gpsimd.dma_start(out=w_sb[d2], in_=w_d[128 * d2:128 * (d2 + 1), :])
                nc.gpsimd.dma_start(out=xqT_sb[d2], in_=xqT_d[128 * d2:128 * (d2 + 1), :])
            nc.gpsimd.dma_start(out=b_sb, in_=b_d[:, :])

            # per-chunk tiles: a chunk is fully written before first read, so
            # tile-granular RAW tracking cannot create emission-order cycles
            kT_c = [[persist.tile([128, 512], F16, name=f"kT{g}_{c}")
                     for c in range(4)] for g in range(2)]
            qT_pad = [persist.tile([128, NQ], F16, name=f"qT{h}") for h in range(H)]
            v_st = [persist.tile([128, H * 33], F16, name=f"vst{s}")
                    for s in range(NKT)]
            outT_c = [[persist.tile([128, 256], F16, name=f"outT{g}_{c}")
                       for c in range(NQC)] for g in range(2)]
            for d2 in range(2):
                nc.gpsimd.dma_start(out=xT_sb[d2], in_=xT_d[128 * d2:128 * (d2 + 1), :])
            for h in range(H):
                nc.gpsimd.memset(qT_pad[h], 0.0)
            for s in range(NKT):
                nc.gpsimd.memset(v_st[s], 1.0)
            for g in range(2):
                nc.gpsimd.dma_start(out=wo_sb[g], in_=wo_d[128 * g:128 * (g + 1), :])

            # ---- projection units (emitted woven into attention) ----
            def qT_unit(hg, c, act_ok=True):
                """q^T for head-group hg, seq chunk c (512 wide)."""
                p = kqps.tile([128, 512], F32, tag="kq", name=f"kqq_{hg}_{c}")
                for d2 in range(2):
                    nc.tensor.matmul(
                        p[:, :], w_sb[d2][:, 128 * hg:128 * (hg + 1)],
                        xqT_sb[d2][:, 512 * c:512 * (c + 1)],
                        start=(d2 == 0), stop=False)
                nc.tensor.matmul(
                    p[:, :], b_sb[:, 128 * hg:128 * (hg + 1)], ones[:, :],
                    start=False, stop=True)
                for j in range(4):
                    dst = qT_pad[4 * hg + j][32 * j:32 * (j + 1),
                                             512 * c:512 * (c + 1)]
                    if act_ok and j % 2 == 1:
                        nc.scalar.activation(out=dst, in_=p[32 * j:32 * (j + 1), :],
                                             func=Copy)
                    else:
                        nc.vector.tensor_copy(out=dst, in_=p[32 * j:32 * (j + 1), :])

            def kT_unit(hg, c, act_ok=True):
                """k^T for head-group hg, seq chunk c (512 wide)."""
                # NOTE: k-bias is omitted: q.(k+b_k) adds a per-query
                # constant over all keys, which cancels exactly in softmax.
                p = kqps.tile([128, 512], F32, tag="kq", name=f"kqk_{hg}_{c}")
                for d2 in range(2):
                    nc.tensor.matmul(
                        p[:, :], w_sb[d2][:, D + 128 * hg:D + 128 * (hg + 1)],
                        xT_sb[d2][:, 512 * c:512 * (c + 1)],
                        start=(d2 == 0), stop=(d2 == 1))
                if act_ok:
                    nc.scalar.activation(out=kT_c[hg][c][:, :], in_=p[:, :],
                                         func=Copy)
                else:
                    nc.vector.tensor_copy(out=kT_c[hg][c][:, :], in_=p[:, :])

            def v_unit(st):
                """v rows for seq tile st (128 wide), all 8 heads + ones col."""
                # NOTE: v-bias is omitted: softmax rows sum to 1, so
                # attn@(v+b_v) = attn@v + b_v; the host adds b_v @ w_out.
                p = kqps.tile([128, D], F32, tag="kq", name=f"vv_{st}")
                for d2 in range(2):
                    nc.tensor.matmul(
                        p[:, :], xT_sb[d2][:, 128 * st:128 * (st + 1)],
                        w_sb[d2][:, 2 * D:3 * D],
                        start=(d2 == 0), stop=(d2 == 1))
                nc.vector.tensor_copy(
                    out=v_st[st].rearrange("p (h c) -> p h c", h=H)[:, :, 0:32],
                    in_=p.rearrange("p (h c) -> p h c", h=H))

            # ---- attention ----
            for hg in range(2):
                av_sb_all = {}
                for qc in range(NQC):
                    av4 = avps.tile([33, 4, 256], F32, tag="av",
                                    name=f"av_{hg}_{qc}")

                    def emit_av(pr, kt):
                        for j in range(4):
                            h = 4 * hg + j
                            # start=True clears has_written for the whole
                            # bank: only the first slice in each bank may
                            # issue it; its sibling inherits the clear.
                            nc.tensor.matmul(
                                av4[:, j, :],
                                v_st[kt][:, 33 * h:33 * h + 33],
                                pr[:, 256 * j:256 * (j + 1)],
                                start=(kt == 0 and j % 2 == 0),
                                stop=(kt == NKT - 1))

                    prev = None
                    for kt in range(NKT):
                        # ---- woven projection filler (PE stays dense) ----
                        if hg == 0 and qc == 0:
                            if kt == 0:
                                qT_unit(0, 0)
                                kT_unit(0, 0)
                            elif kt == 1:
                                qT_unit(0, 1)
                            elif kt % 4 == 0:
                                kT_unit(0, kt // 4)
                            v_unit(kt)
                        elif hg == 0 and qc == 1:
                            if kt in (0, 4):
                                qT_unit(1, kt // 4, act_ok=False)
                            elif kt in (8, 12):
                                kT_unit(1, (kt - 8) // 4, act_ok=False)
                        elif hg == 0 and qc == 2 and kt in (0, 4):
                            kT_unit(1, 2 + kt // 4, act_ok=False)

                        S = scps.tile([128, 4, 256], F32, tag="S",
                                      name=f"S_{hg}_{qc}_{kt}")
                        for j in range(4):
                            nc.tensor.matmul(
                                S[:, j, :],
                                kT_c[hg][kt // 4][:, 128 * (kt % 4):128 * (kt % 4 + 1)],
                                qT_pad[4 * hg + j][:, QC * qc:QC * (qc + 1)],
                                start=True, stop=True)
                        pr = prpool.tile([128, 4 * QC], F16, tag="pr",
                                         name=f"pr_{hg}_{qc}_{kt}")
                        nc.scalar.activation(
                            out=pr, in_=S.rearrange("p a b -> p (a b)"),
                            func=Exp, scale=SCALE)
                        if prev is not None:
                            emit_av(prev, kt - 1)
                        prev = pr
                    emit_av(prev, NKT - 1)

                    a = avsb_pool.tile([33, 4, 256], F32, tag="avsb",
                                       name=f"avsb_{hg}_{qc}")
                    nc.vector.tensor_copy(a, av4[:, :, :])
                    nc.sync.dma_start(out=den_dram[hg, qc, :, :],
                                      in_=a[32:33, :, :])

                    # per-qc normalize: batched reciprocal [128, 8], one
                    # 4-head broadcast read, 4 muls (+ outproj when hg==1)
                    denb = norm_pool.tile([128, 8], F32, tag="denb",
                                          name=f"denb{hg}_{qc}")
                    nc.sync.dma_start(
                        out=denb,
                        in_=den_dram[hg, qc, :, :].rearrange("a c -> (a c)")
                        .rearrange("(p f) -> p f", p=128))
                    recb = norm_pool.tile([128, 8], F32, tag="recb",
                                          name=f"recb{hg}_{qc}")
                    nc.vector.reciprocal(recb, denb)
                    nc.sync.dma_start(
                        out=recip_dram[hg, qc, :, :].rearrange("a c -> (a c)")
                        .rearrange("(p f) -> p f", p=128),
                        in_=recb)
                    for p2 in range(2):
                        row = recip_dram[hg, qc, p2, :]
                        bc = norm_pool.tile([32, 512], F32, tag="bc",
                                            name=f"bc_{hg}_{qc}_{p2}")
                        nc.gpsimd.dma_start(
                            out=bc,
                            in_=bass.AP(tensor=row.tensor, offset=row.offset,
                                        ap=[[0, 32], row.ap[-1]]))
                        for i2 in range(2):
                            j = 2 * p2 + i2
                            nc.vector.tensor_mul(
                                outT_c[hg][qc][32 * j:32 * (j + 1), :],
                                a[0:32, j, :],
                                bc[:, 256 * i2:256 * (i2 + 1)])
                    if hg == 1:
                        for qt in (2 * qc, 2 * qc + 1):
                            po = kqps.tile([128, D], F32, tag="kq", name=f"po{qt}")
                            for g in range(2):
                                nc.tensor.matmul(
                                    po[:, :],
                                    outT_c[g][qt // 2][:, 128 * (qt % 2):128 * (qt % 2 + 1)],
                                    wo_sb[g][:, :],
                                    start=(g == 0), stop=(g == 1))
                            o = out_pool.tile([128, D], F32, tag="o", name=f"o{qt}")
                            nc.vector.tensor_copy(o, po[:, :])
                            nc.sync.dma_start(
                                out=out_d[128 * qt:128 * (qt + 1), :], in_=o)

    nc.compile()
    _BUILD_CACHE["nc"] = nc
    return nc


def _run(x, w_qkv, b_qkv, w_out, trace=False):
    nc = build()
    in_maps = []
    for c in range(8):
        bi, qh = c // 2, c % 2
        in_maps.append({
            "xT": np.ascontiguousarray(x[bi].T),
            "xqT": np.ascontiguousarray(x[bi, NQ * qh:NQ * (qh + 1)].T),
            "w_qkv": np.ascontiguousarray(w_qkv),
            "b_qkv": np.ascontiguousarray(b_qkv.reshape(1, 3 * D)),
            "w_out": np.ascontiguousarray(w_out),
        })
    res = run_bass_kernel_spmd(nc, in_maps, core_ids=list(range(8)), trace=trace)
    out = np.empty((B, N, D), dtype=np.float32)
    for c in range(8):
        bi, qh = c // 2, c % 2
        out[bi, NQ * qh:NQ * (qh + 1)] = res.results[c]["out"]
    # v-bias correction (exact): attn@(v+b_v) = attn@v + b_v, so the device
    # omits b_v and the host adds its image through the output projection.
    bv = np.asarray(b_qkv, np.float32).reshape(-1)[2 * D:3 * D]
    out += (bv @ np.asarray(w_out, np.float32))[None, None, :]
    return out, res


def kernel(x, w_qkv, b_qkv, w_out, b_out):
    x = np.asarray(x, dtype=np.float32)
    out, _ = _run(x, np.asarray(w_qkv, np.float32), np.asarray(b_qkv, np.float32),
                  np.asarray(w_out, np.float32))
    return out + np.asarray(b_out, np.float32)[None, None, :]



# revision 4
# speedup vs baseline: 1.0330x; 1.0330x over previous
"""Multi-head attention block (b=4, n=2048, d=256, h=8) on 8 TRN2 NeuronCores.

Sharding: core c handles (batch bi=c//2, query-half qh=c%2): it computes
K/V for the full sequence of its batch and Q for its 1024-row query half,
producing 1024 complete rows of the final output (host concatenates and
adds b_out; no cross-core reduction).

Design (matmul operands fp16; PSUM fp32). Two phases:

Phase 1 (projections, PE dense): all of Q^T (zero-padded per head to 128
partitions), K^T (4 heads stacked per 128 partitions), and [v|ones]
(denominator folded as row 32) are produced up front; psum->SBUF copies
round-robin across the ACT/DVE/Pool engines.

Phase 2 (attention): for each (head-group, q-chunk of 256): 16 k-tiles:
scores S[128,4,256] (4 matmuls, K=128, the qT_pad zero rows mask other
heads), then exp, then AV accumulate into av[33,4,256]. Two key changes
vs the 206us version:
  - exp is SPLIT across engines: 11/16 k-tiles get exact Exp on the
    ACT engine; 5/16 get a Schraudolph int16 exp on DVE/Pool
    (i16 = rint(dots*SCALE*1024/ln2 + (15*1024 - C*1024)); bitcast fp16
    ~= exp(dots*SCALE) with ~2.7% sawtooth error, C tuned zero-mean).
    This removes ACT (148us busy at baseline) from the critical path;
    measured end-to-end rel-err contribution ~7e-3 (budget 2e-2).
  - AV lags S by TWO k-tiles (was one), so the PE never waits on the
    exp engines' ~1.2us latency (PE period/k-tile 854ns, slack 2x854).
PSUM: S 2x2 banks + av 2 + proj/outproj 2 = 8 banks.
Normalization: denominator rows -> DRAM bounce -> batched reciprocal
[128,8] per q-chunk -> broadcast-read -> DVE multiply -> outT (fp16);
output projection emitted per q-chunk as soon as outT is ready.
Host: uploads fp16 inputs (halves DMA), adds b_out and the exact v-bias
image b_v @ w_out (softmax rows sum to 1, so attn@(v+b_v) = attn@v + b_v).
"""
import numpy as np

import concourse.bacc as bacc
import concourse.bass as bass
import concourse.mybir as mybir
import concourse.tile as tile
from concourse.bass_utils import run_bass_kernel_spmd

F32 = mybir.dt.float32
F16 = mybir.dt.float16
I16 = mybir.dt.int16
Exp = mybir.ActivationFunctionType.Exp
Copy = mybir.ActivationFunctionType.Copy
MUL = mybir.AluOpType.mult
ADD = mybir.AluOpType.add

B, N, D = 4, 2048, 256
H, DH = 8, 32
NQ = N // 2            # per-core query rows
SCALE = D ** -0.5      # 0.0625
NKT = N // 128         # 16 k-tiles
QC = 256               # q-chunk
NQC = NQ // QC         # 4 q-chunks per core

LN2 = float(np.log(2.0))
HACK_C = 0.0573        # zero-mean shift for the Schraudolph sawtooth
HACK_A = SCALE * 1024.0 / LN2
HACK_B = 15.0 * 1024.0 - HACK_C * 1024.0
# engine per k-tile: exact ACT exp, or int16-hack on DVE (Pool cannot
# read PSUM -- BIR verifier rejects GPSIMD PSUM access)
ENGMAP = {2: "dve", 5: "dve", 8: "dve", 11: "dve", 14: "dve"}

_BUILD_CACHE = {}


def build():
    if "nc" in _BUILD_CACHE:
        return _BUILD_CACHE["nc"]
    nc = bacc.Bacc()

    xT_d = nc.dram_tensor("xT", [D, N], F16, kind="ExternalInput")
    xqT_d = nc.dram_tensor("xqT", [D, NQ], F16, kind="ExternalInput")
    w_d = nc.dram_tensor("w_qkv", [D, 3 * D], F16, kind="ExternalInput")
    b_d = nc.dram_tensor("b_qkv", [1, 3 * D], F16, kind="ExternalInput")
    wo_d = nc.dram_tensor("w_out", [D, D], F16, kind="ExternalInput")
    out_d = nc.dram_tensor("out", [NQ, D], F32, kind="ExternalOutput")
    den_dram = nc.dram_tensor("den_scratch", [2, NQC, 2, 512], F32)
    recip_dram = nc.dram_tensor("recip_scratch", [2, NQC, 2, 512], F32)

    with tile.TileContext(nc) as tc:
        with (
            tc.tile_pool(name="persist", bufs=1) as persist,
            tc.tile_pool(name="probs", bufs=4) as prpool,
            tc.tile_pool(name="hackt", bufs=3) as tpool,
            tc.tile_pool(name="avsb", bufs=3) as avsb_pool,
            tc.tile_pool(name="norm", bufs=4) as norm_pool,
            tc.tile_pool(name="outsb", bufs=3) as out_pool,
            tc.tile_pool(name="kqps", bufs=2, space="PSUM") as kqps,
            tc.tile_pool(name="scps", bufs=2, space="PSUM") as scps,
            tc.tile_pool(name="avps", bufs=1, space="PSUM") as avps,
        ):
            # ---- persistent tiles / loads ----
            ones = persist.tile([1, 512], F16, name="ones")
            nc.vector.memset(ones, 1.0)

            w_sb = [persist.tile([128, 3 * D], F16, name=f"w{d2}") for d2 in range(2)]
            b_sb = persist.tile([1, 3 * D], F16, name="b_sb")
            xT_sb = [persist.tile([128, N], F16, name=f"xT{d2}") for d2 in range(2)]
            xqT_sb = [persist.tile([128, NQ], F16, name=f"xq{d2}") for d2 in range(2)]
            wo_sb = [persist.tile([128, D], F16, name=f"wo{g}") for g in range(2)]
            for d2 in range(2):
                nc.sync.dma_start(out=w_sb[d2], in_=w_d[128 * d2:128 * (d2 + 1), :])
                nc.sync.dma_start(out=xqT_sb[d2], in_=xqT_d[128 * d2:128 * (d2 + 1), :])
                nc.sync.dma_start(out=xT_sb[d2], in_=xT_d[128 * d2:128 * (d2 + 1), :])
            nc.sync.dma_start(out=b_sb, in_=b_d[:, :])
            for g in range(2):
                nc.sync.dma_start(out=wo_sb[g], in_=wo_d[128 * g:128 * (g + 1), :])

            kT_c = [[persist.tile([128, 512], F16, name=f"kT{g}_{c}")
                     for c in range(4)] for g in range(2)]
            qT_pad = [persist.tile([128, NQ], F16, name=f"qT{h}") for h in range(H)]
            v_st = [persist.tile([128, H * 33], F16, name=f"vst{s}")
                    for s in range(NKT)]
            outT_c = [[persist.tile([128, 256], F16, name=f"outT{g}_{c}")
                       for c in range(NQC)] for g in range(2)]
            for h in range(H):
                nc.gpsimd.memset(qT_pad[h], 0.0)
            for s in range(NKT):
                nc.gpsimd.memset(v_st[s], 1.0)

            # psum->SBUF copy engine rotation (ACT / DVE)
            _cp = [0]

            def copy(out, in_):
                _cp[0] = (_cp[0] + 1) % 2
                if _cp[0] == 0:
                    nc.scalar.activation(out=out, in_=in_, func=Copy)
                else:
                    nc.vector.tensor_copy(out=out, in_=in_)

            # ---- phase 1: projections (PE dense, copies on ACT/DVE) ----
            def qT_unit(hg, c):
                """q^T for head-group hg, seq chunk c (512 wide)."""
                p = kqps.tile([128, 512], F32, tag="kq", name=f"kqq_{hg}_{c}")
                for d2 in range(2):
                    nc.tensor.matmul(
                        p[:, :], w_sb[d2][:, 128 * hg:128 * (hg + 1)],
                        xqT_sb[d2][:, 512 * c:512 * (c + 1)],
                        start=(d2 == 0), stop=False)
                nc.tensor.matmul(
                    p[:, :], b_sb[:, 128 * hg:128 * (hg + 1)], ones[:, :],
                    start=False, stop=True)
                for j in range(4):
                    copy(qT_pad[4 * hg + j][32 * j:32 * (j + 1),
                                            512 * c:512 * (c + 1)],
                         p[32 * j:32 * (j + 1), :])

            def kT_unit(hg, c):
                """k^T for head-group hg, seq chunk c (512 wide).
                k-bias omitted: q.(k+b_k) adds a per-query constant over all
                keys, which cancels exactly in softmax."""
                p = kqps.tile([128, 512], F32, tag="kq", name=f"kqk_{hg}_{c}")
                for d2 in range(2):
                    nc.tensor.matmul(
                        p[:, :], w_sb[d2][:, D + 128 * hg:D + 128 * (hg + 1)],
                        xT_sb[d2][:, 512 * c:512 * (c + 1)],
                        start=(d2 == 0), stop=(d2 == 1))
                copy(kT_c[hg][c][:, :], p[:, :])

            def v_unit(st):
                """v rows for seq tile st (128 wide), all 8 heads + ones col.
                v-bias omitted: softmax rows sum to 1, so attn@(v+b_v) =
                attn@v + b_v; the host adds b_v @ w_out."""
                p = kqps.tile([128, D], F32, tag="kq", name=f"vv_{st}")
                for d2 in range(2):
                    nc.tensor.matmul(
                        p[:, :], xT_sb[d2][:, 128 * st:128 * (st + 1)],
                        w_sb[d2][:, 2 * D:3 * D],
                        start=(d2 == 0), stop=(d2 == 1))
                copy(v_st[st].rearrange("p (h c) -> p h c", h=H)[:, :, 0:32],
                     p.rearrange("p (h c) -> p h c", h=H))

            # interleave units so the 2-deep kq psum ring never stalls PE
            units = []
            for hg in range(2):
                for c in range(2):
                    units.append(lambda hg=hg, c=c: qT_unit(hg, c))
            for hg in range(2):
                for c in range(4):
                    units.append(lambda hg=hg, c=c: kT_unit(hg, c))
            for st in range(NKT):
                units.append(lambda st=st: v_unit(st))
            for u in units:
                u()

            # ---- phase 2: attention ----
            for hg in range(2):
                for qc in range(NQC):
                    av4 = avps.tile([33, 4, 256], F32, tag="av",
                                    name=f"av_{hg}_{qc}")

                    def emit_av(pr, kt):
                        for j in range(4):
                            h = 4 * hg + j
                            # start=True clears has_written for the whole
                            # bank: only the first slice in each bank may
                            # issue it; its sibling inherits the clear.
                            nc.tensor.matmul(
                                av4[:, j, :],
                                v_st[kt][:, 33 * h:33 * h + 33],
                                pr[:, 256 * j:256 * (j + 1)],
                                start=(kt == 0 and j % 2 == 0),
                                stop=(kt == NKT - 1))

                    hist = {}
                    for kt in range(NKT):
                        S = scps.tile([128, 4, 256], F32, tag="S",
                                      name=f"S_{hg}_{qc}_{kt}")
                        for j in range(4):
                            nc.tensor.matmul(
                                S[:, j, :],
                                kT_c[hg][kt // 4][:, 128 * (kt % 4):128 * (kt % 4 + 1)],
                                qT_pad[4 * hg + j][:, QC * qc:QC * (qc + 1)],
                                start=True, stop=True)
                        eng = ENGMAP.get(kt, "act")
                        if eng == "act":
                            pr = prpool.tile([128, 4 * QC], F16, tag="pr",
                                             name=f"pr_{hg}_{qc}_{kt}")
                            nc.scalar.activation(
                                out=pr, in_=S.rearrange("p a b -> p (a b)"),
                                func=Exp, scale=SCALE)
                        else:
                            t = tpool.tile([128, 4 * QC], I16, tag="t",
                                           name=f"t_{hg}_{qc}_{kt}")
                            e = nc.vector if eng == "dve" else nc.gpsimd
                            e.tensor_scalar(
                                out=t, in0=S.rearrange("p a b -> p (a b)"),
                                scalar1=HACK_A, scalar2=HACK_B,
                                op0=MUL, op1=ADD)
                            pr = t.bitcast(F16)
                        hist[kt] = pr
                        if kt >= 2:
                            emit_av(hist.pop(kt - 2), kt - 2)
                    emit_av(hist.pop(NKT - 2), NKT - 2)
                    emit_av(hist.pop(NKT - 1), NKT - 1)

                    a = avsb_pool.tile([33, 4, 256], F32, tag="avsb",
                                       name=f"avsb_{hg}_{qc}")
                    nc.vector.tensor_copy(a, av4[:, :, :])
                    nc.sync.dma_start(out=den_dram[hg, qc, :, :],
                                      in_=a[32:33, :, :])

                    # per-qc normalize: batched reciprocal [128, 8], one
                    # 4-head broadcast read, 4 muls (+ outproj when hg==1)
                    denb = norm_pool.tile([128, 8], F32, tag="denb",
                                          name=f"denb{hg}_{qc}")
                    nc.sync.dma_start(
                        out=denb,
                        in_=den_dram[hg, qc, :, :].rearrange("a c -> (a c)")
                        .rearrange("(p f) -> p f", p=128))
                    recb = norm_pool.tile([128, 8], F32, tag="recb",
                                          name=f"recb{hg}_{qc}")
                    nc.vector.reciprocal(recb, denb)
                    nc.sync.dma_start(
                        out=recip_dram[hg, qc, :, :].rearrange("a c -> (a c)")
                        .rearrange("(p f) -> p f", p=128),
                        in_=recb)
                    for p2 in range(2):
                        row = recip_dram[hg, qc, p2, :]
                        bc = norm_pool.tile([32, 512], F32, tag="bc",
                                            name=f"bc_{hg}_{qc}_{p2}")
                        nc.gpsimd.dma_start(
                            out=bc,
                            in_=bass.AP(tensor=row.tensor, offset=row.offset,
                                        ap=[[0, 32], row.ap[-1]]))
                        for i2 in range(2):
                            j = 2 * p2 + i2
                            # SBUF-only, so it can run on Pool (DVE is busy
                            # with the int16-exp tiles)
                            nc.gpsimd.tensor_mul(
                                outT_c[hg][qc][32 * j:32 * (j + 1), :],
                                a[0:32, j, :],
                                bc[:, 256 * i2:256 * (i2 + 1)])
                    if hg == 1:
                        for qt in (2 * qc, 2 * qc + 1):
                            po = kqps.tile([128, D], F32, tag="kq", name=f"po{qt}")
                            for g in range(2):
                                nc.tensor.matmul(
                                    po[:, :],
                                    outT_c[g][qt // 2][:, 128 * (qt % 2):128 * (qt % 2 + 1)],
                                    wo_sb[g][:, :],
                                    start=(g == 0), stop=(g == 1))
                            o = out_pool.tile([128, D], F32, tag="o", name=f"o{qt}")
                            nc.vector.tensor_copy(o, po[:, :])
                            nc.sync.dma_start(
                                out=out_d[128 * qt:128 * (qt + 1), :], in_=o)

    nc.compile()
    _BUILD_CACHE["nc"] = nc
    return nc


def _run(x, w_qkv, b_qkv, w_out, trace=False):
    nc = build()
    x16 = np.asarray(x, np.float16)
    w16 = np.ascontiguousarray(np.asarray(w_qkv, np.float16))
    b16 = np.ascontiguousarray(np.asarray(b_qkv, np.float16).reshape(1, 3 * D))
    wo16 = np.ascontiguousarray(np.asarray(w_out, np.float16))
    in_maps = []
    for c in range(8):
        bi, qh = c // 2, c % 2
        in_maps.append({
            "xT": np.ascontiguousarray(x16[bi].T),
            "xqT": np.ascontiguousarray(x16[bi, NQ * qh:NQ * (qh + 1)].T),
            "w_qkv": w16,
            "b_qkv": b16,
            "w_out": wo16,
        })
    res = run_bass_kernel_spmd(nc, in_maps, core_ids=list(range(8)), trace=trace)
    out = np.empty((B, N, D), dtype=np.float32)
    for c in range(8):
        bi, qh = c // 2, c % 2
        out[bi, NQ * qh:NQ * (qh + 1)] = res.results[c]["out"]
    # v-bias correction (exact): attn@(v+b_v) = attn@v + b_v, so the device
    # omits b_v and the host adds its image through the output projection.
    bv = np.asarray(b_qkv, np.float32).reshape(-1)[2 * D:3 * D]
    out += (bv @ np.asarray(w_out, np.float32))[None, None, :]
    return out, res


def kernel(x, w_qkv, b_qkv, w_out, b_out):
    x = np.asarray(x, dtype=np.float32)
    out, _ = _run(x, np.asarray(w_qkv, np.float32), np.asarray(b_qkv, np.float32),
                  np.asarray(w_out, np.float32))
    return out + np.asarray(b_out, np.float32)[None, None, :]


# revision 13
# speedup vs baseline: 1.1383x; 1.1020x over previous
"""Multi-head attention block (b=4, n=2048, d=256, h=8) on 8 TRN2 NeuronCores.

Sharding: core c handles (batch bi=c//2, query-half qh=c%2): it computes
K/V for the full sequence of its batch and Q for its 1024-row query half,
producing 1024 complete rows of the final output (host concatenates and
adds b_out; no cross-core reduction).

Design (matmul operands fp16; PSUM fp32). Attention runs per
(head-group of 4, q-chunk of 256) over 16 k-tiles of 128 keys:

  - Scores: TWO matmuls per k-tile (a matmul's output must fit one
    PSUM bank = 512 fp32): lhsT = 4-heads-stacked kT [128,128], rhs =
    qT_q[hg][qc] [128, (4 heads, 256 q)] halves; each (head j, q)
    column is zero-padded outside rows 32j..32j+32 so the stacked kT
    is masked per column. S psum [128, 2, 512].
  - exp SPLIT across engines: 10/16 k-tiles exact Exp on ACT; 6/16 a
    Schraudolph int16 exp on DVE (i16 = rint(dots*SCALE*1024/ln2 + B);
    bitcast fp16 ~ exp, ~2.7% sawtooth, C tuned zero-mean; measured
    end-to-end rel-err ~7.5e-3 vs 2e-2 budget). ACT alone (148us busy)
    was the co-bottleneck of the 206us version.
  - AV: TWO matmuls per k-tile (pairs of heads): lhsT = [v_h|v_h'|1]
    [128, 65] (halves LDWEIGHTS vs per-head; the walrus build disables
    ldw-opt so every matmul reloads weights and the PE sequencer was
    near-saturated at 8 ldweights/k-tile). Row 64 of the psum = the
    softmax denominators of BOTH pair members (ones column x probs);
    member e's values sit 32-aligned at rows 32e..32e+31 (engine APs
    must start at 32-aligned partitions). Off-diagonal (head x other
    member's probs) blocks are dead values.
    av2 psum [65, 2, 512]: pair p accumulates its own bank cleanly.
  - AV lags S by TWO k-tiles so the PE never waits on exp latency
    (PE period 854ns/k-tile, exp ~1.2us).
  - Projections (Q^T padded, kT stacked, [v|1]) are WOVEN into the
    first attention iterations >=2 k-tiles ahead of use; x is DMA'd in
    512-column chunks so the first units start after ~0.4MB.
  - Normalize per (hg,qc) with NO DRAM bounce: the den row is
    SBUF->SBUF DMA'd into [128, 8] for one exact DVE reciprocal, DMA'd
    back to a row, gpsimd partition_broadcast to 32 rows, 4 Pool
    multiplies -> outT fp16. (reciprocal_approx_* custom-DVE ops
    compute garbage in this environment -- validated on HW.)
    Output projection is deferred into the NEXT chunk's stream so the
    PE never waits on the normalize chain.
  - PSUM: S 2x2 banks + av2 2 + proj/outproj 2 = 8 banks.

Host: uploads fp16 inputs (halves DMA), adds b_out and the exact v-bias
image b_v @ w_out (softmax rows sum to 1 => attn@(v+b_v) = attn@v+b_v);
k-bias drops (adds a per-query constant, cancels in softmax).
"""
import numpy as np

import concourse.bacc as bacc
import concourse.bass as bass
import concourse.mybir as mybir
import concourse.tile as tile
from concourse.bass_utils import run_bass_kernel_spmd

F32 = mybir.dt.float32
F16 = mybir.dt.float16
I16 = mybir.dt.int16
Exp = mybir.ActivationFunctionType.Exp
Copy = mybir.ActivationFunctionType.Copy
MUL = mybir.AluOpType.mult
ADD = mybir.AluOpType.add

B, N, D = 4, 2048, 256
H, DH = 8, 32
NQ = N // 2            # per-core query rows
SCALE = D ** -0.5      # 0.0625
NKT = N // 128         # 16 k-tiles
QC = 256               # q-chunk
NQC = NQ // QC         # 4 q-chunks per core

LN2 = float(np.log(2.0))
HACK_C = 0.0573        # zero-mean shift for the Schraudolph sawtooth
HACK_A = SCALE * 1024.0 / LN2
HACK_B = 15.0 * 1024.0 - HACK_C * 1024.0
# k-tiles whose exp runs as the int16 hack on DVE (Pool cannot read PSUM);
# none in the first two k-tiles: DVE is still normalizing the previous chunk
HACK_KT = (3, 5, 8, 10, 12, 14)

_BUILD_CACHE = {}


def build():
    if "nc" in _BUILD_CACHE:
        return _BUILD_CACHE["nc"]
    nc = bacc.Bacc()

    xT_d = nc.dram_tensor("xT", [D, N], F16, kind="ExternalInput")
    xqT_d = nc.dram_tensor("xqT", [D, NQ], F16, kind="ExternalInput")
    w_d = nc.dram_tensor("w_qkv", [D, 3 * D], F16, kind="ExternalInput")
    b_d = nc.dram_tensor("b_qkv", [1, 3 * D], F16, kind="ExternalInput")
    wo_d = nc.dram_tensor("w_out", [D, D], F16, kind="ExternalInput")
    out_d = nc.dram_tensor("out", [NQ, D], F32, kind="ExternalOutput")

    with tile.TileContext(nc) as tc:
        with (
            tc.tile_pool(name="persist", bufs=1) as persist,
            tc.tile_pool(name="probs", bufs=4) as prpool,
            tc.tile_pool(name="hackt", bufs=4) as tpool,
            tc.tile_pool(name="avsb", bufs=2) as avsb_pool,
            tc.tile_pool(name="norm", bufs=4) as norm_pool,
            tc.tile_pool(name="outsb", bufs=3) as out_pool,
            tc.tile_pool(name="kqps", bufs=2, space="PSUM") as kqps,
            tc.tile_pool(name="scps", bufs=2, space="PSUM") as scps,
            tc.tile_pool(name="avps", bufs=1, space="PSUM") as avps,
        ):
            # ---- persistent tiles ----
            ones = persist.tile([1, 512], F16, name="ones")
            nc.vector.memset(ones, 1.0)

            w_sb = [persist.tile([128, 3 * D], F16, name=f"w{d2}") for d2 in range(2)]
            b_sb = persist.tile([1, 3 * D], F16, name="b_sb")
            # x chunks [128, 512] so the first units start after ~0.4MB of DMA
            xT_sb = [[persist.tile([128, 512], F16, name=f"xT{d2}_{c}")
                      for c in range(4)] for d2 in range(2)]
            xqT_sb = [[persist.tile([128, 512], F16, name=f"xq{d2}_{c}")
                       for c in range(2)] for d2 in range(2)]
            wo_sb = [persist.tile([128, D], F16, name=f"wo{g}") for g in range(2)]

            for d2 in range(2):
                nc.sync.dma_start(out=w_sb[d2], in_=w_d[128 * d2:128 * (d2 + 1), :])
            nc.sync.dma_start(out=b_sb, in_=b_d[:, :])
            for d2 in range(2):
                nc.sync.dma_start(out=xqT_sb[d2][0],
                                  in_=xqT_d[128 * d2:128 * (d2 + 1), 0:512])
            for c in range(4):
                for d2 in range(2):
                    nc.sync.dma_start(
                        out=xT_sb[d2][c],
                        in_=xT_d[128 * d2:128 * (d2 + 1), 512 * c:512 * (c + 1)])
            for d2 in range(2):
                nc.sync.dma_start(out=xqT_sb[d2][1],
                                  in_=xqT_d[128 * d2:128 * (d2 + 1), 512:1024])
            for g in range(2):
                nc.sync.dma_start(out=wo_sb[g], in_=wo_d[128 * g:128 * (g + 1), :])

            kT_c = [[persist.tile([128, 512], F16, name=f"kT{g}_{c}")
                     for c in range(4)] for g in range(2)]
            # per-(hg,qc) padded q: column (j, q) nonzero only rows 32j..32j+32
            qT_q = [[persist.tile([128, 4, QC], F16, name=f"qTq{g}_{c}")
                     for c in range(NQC)] for g in range(2)]
            # per k-tile: 4 head-pairs x [v_h(32) | v_h'(32) | ones] = 65 cols
            v_st = [persist.tile([128, 4 * 65], F16, name=f"vst{s}")
                    for s in range(NKT)]
            outT_c = [[persist.tile([128, 256], F16, name=f"outT{g}_{c}")
                       for c in range(NQC)] for g in range(2)]
            for g in range(2):
                for c in range(NQC):
                    nc.gpsimd.memset(qT_q[g][c], 0.0)
            for s in range(NKT):
                nc.gpsimd.memset(v_st[s], 1.0)

            # psum->SBUF copy engine rotation (ACT / DVE; Pool cannot read PSUM)
            _cp = [0]

            def copy(out, in_):
                _cp[0] = (_cp[0] + 1) % 2
                if _cp[0] == 0:
                    nc.scalar.activation(out=out, in_=in_, func=Copy)
                else:
                    nc.vector.tensor_copy(out=out, in_=in_)

            # ---- projection units (woven into the attention stream) ----
            def qT_unit(hg, c):
                """q^T for head-group hg, 512 q columns (q-chunks 2c, 2c+1)."""
                p = kqps.tile([128, 512], F32, tag="kq", name=f"kqq_{hg}_{c}")
                for d2 in range(2):
                    nc.tensor.matmul(
                        p[:, :], w_sb[d2][:, 128 * hg:128 * (hg + 1)],
                        xqT_sb[d2][c],
                        start=(d2 == 0), stop=False)
                nc.tensor.matmul(
                    p[:, :], b_sb[:, 128 * hg:128 * (hg + 1)], ones[:, :],
                    start=False, stop=True)
                for j in range(4):
                    for half in range(2):
                        copy(qT_q[hg][2 * c + half][32 * j:32 * (j + 1), j, :],
                             p[32 * j:32 * (j + 1), 256 * half:256 * (half + 1)])

            def kT_unit(hg, c):
                """k^T for head-group hg, seq chunk c (512 wide)."""
                p = kqps.tile([128, 512], F32, tag="kq", name=f"kqk_{hg}_{c}")
                for d2 in range(2):
                    nc.tensor.matmul(
                        p[:, :], w_sb[d2][:, D + 128 * hg:D + 128 * (hg + 1)],
                        xT_sb[d2][c],
                        start=(d2 == 0), stop=(d2 == 1))
                copy(kT_c[hg][c][:, :], p[:, :])

            def v_unit(st):
                """v rows for seq tile st (128 wide), all 8 heads + ones col."""
                p = kqps.tile([128, D], F32, tag="kq", name=f"vv_{st}")
                for d2 in range(2):
                    nc.tensor.matmul(
                        p[:, :], xT_sb[d2][st // 4][:, 128 * (st % 4):128 * (st % 4 + 1)],
                        w_sb[d2][:, 2 * D:3 * D],
                        start=(d2 == 0), stop=(d2 == 1))
                copy(v_st[st].rearrange("p (pp s) -> p pp s", s=65)[:, :, 0:64],
                     p.rearrange("p (pp c) -> p pp c", pp=4))

            # weave schedule: units emitted >=2 k-tiles before first use
            weave = {}
            weave[(0, 0, 0)] = [lambda: v_unit(2)]
            weave[(0, 0, 1)] = [lambda: v_unit(3), lambda: kT_unit(0, 1)]
            for st in range(4, NKT):
                weave.setdefault((0, 0, st - 2), []).append(
                    lambda st=st: v_unit(st))
            weave.setdefault((0, 0, 3), []).append(lambda: kT_unit(0, 2))
            weave.setdefault((0, 0, 7), []).append(lambda: kT_unit(0, 3))
            weave.setdefault((0, 0, 9), []).append(lambda: qT_unit(0, 1))
            weave[(0, 1, 0)] = [lambda: qT_unit(1, 0)]
            weave[(0, 1, 2)] = [lambda: kT_unit(1, 0)]
            weave[(0, 1, 5)] = [lambda: kT_unit(1, 1)]
            weave[(0, 2, 0)] = [lambda: kT_unit(1, 2)]
            weave[(0, 2, 3)] = [lambda: kT_unit(1, 3)]
            weave[(0, 2, 6)] = [lambda: qT_unit(1, 1)]

            # prefix: just enough for (hg0, qc0..1) k-tiles 0..3
            qT_unit(0, 0)
            kT_unit(0, 0)
            v_unit(0)
            v_unit(1)

            # ---- attention ----
            deferred_outproj = []

            def emit_outproj(qc):
                for qt in (2 * qc, 2 * qc + 1):
                    po = kqps.tile([128, D], F32, tag="kq", name=f"po{qt}")
                    for g in range(2):
                        nc.tensor.matmul(
                            po[:, :],
                            outT_c[g][qt // 2][:, 128 * (qt % 2):128 * (qt % 2 + 1)],
                            wo_sb[g][:, :],
                            start=(g == 0), stop=(g == 1))
                    o = out_pool.tile([128, D], F32, tag="o", name=f"o{qt}")
                    copy(o, po[:, :])
                    nc.sync.dma_start(out=out_d[128 * qt:128 * (qt + 1), :], in_=o)

            for hg in range(2):
                for qc in range(NQC):
                    av2 = avps.tile([65, 2, 512], F32, tag="av",
                                    name=f"av_{hg}_{qc}")

                    def emit_av(pr, kt):
                        for p in range(2):
                            pp = 2 * hg + p
                            nc.tensor.matmul(
                                av2[:, p, :],
                                v_st[kt][:, 65 * pp:65 * pp + 65],
                                pr[:, 512 * p:512 * (p + 1)],
                                start=(kt == 0), stop=(kt == NKT - 1))

                    hist = {}
                    for kt in range(NKT):
                        for u in weave.get((hg, qc, kt), ()):
                            u()
                        if deferred_outproj and kt == 5:
                            emit_outproj(deferred_outproj.pop())
                        S = scps.tile([128, 2, 512], F32, tag="S",
                                      name=f"S_{hg}_{qc}_{kt}")
                        for p in range(2):
                            nc.tensor.matmul(
                                S[:, p, :],
                                kT_c[hg][kt // 4][:, 128 * (kt % 4):128 * (kt % 4 + 1)],
                                qT_q[hg][qc].rearrange("p a b -> p (a b)")[:, 512 * p:512 * (p + 1)],
                                start=True, stop=True)
                        if kt not in HACK_KT:
                            pr = prpool.tile([128, 4 * QC], F16, tag="pr",
                                             name=f"pr_{hg}_{qc}_{kt}")
                            nc.scalar.activation(
                                out=pr, in_=S.rearrange("p a b -> p (a b)"),
                                func=Exp, scale=SCALE)
                        else:
                            t = tpool.tile([128, 4 * QC], I16, tag="t",
                                           name=f"t_{hg}_{qc}_{kt}")
                            nc.vector.tensor_scalar(
                                out=t, in0=S.rearrange("p a b -> p (a b)"),
                                scalar1=HACK_A, scalar2=HACK_B,
                                op0=MUL, op1=ADD)
                            pr = t.bitcast(F16)
                        hist[kt] = pr
                        if kt >= 2:
                            emit_av(hist.pop(kt - 2), kt - 2)
                    emit_av(hist.pop(NKT - 2), NKT - 2)
                    emit_av(hist.pop(NKT - 1), NKT - 1)

                    # normalize: row 64 of av2 = denominators of BOTH pair
                    # members (ones column): den[j=2p+e, q] = a[64, p, 256e+q]
                    a = avsb_pool.tile([65, 2, 512], F32, tag="avsb",
                                       name=f"avsb_{hg}_{qc}")
                    nc.vector.tensor_copy(a, av2[:, :, :])
                    denb = norm_pool.tile([128, 8], F32, tag="denb",
                                          name=f"denb{hg}_{qc}")
                    nc.sync.dma_start(out=denb, in_=a[64:65, :, :])
                    recb = norm_pool.tile([128, 8], F32, tag="recb",
                                          name=f"recb{hg}_{qc}")
                    nc.vector.reciprocal(recb, denb)
                    rb = norm_pool.tile([1, 2, 512], F32, tag="rb",
                                        name=f"rb{hg}_{qc}")
                    nc.sync.dma_start(out=rb, in_=recb)
                    # 64 partitions so each mul's two SBUF inputs share a
                    # base partition (in0 at 32e must equal in1's base)
                    bc = norm_pool.tile([64, 2, 512], F32, tag="bc",
                                        name=f"bc_{hg}_{qc}")
                    nc.gpsimd.partition_broadcast(
                        bc.rearrange("p a b -> p (a b)"),
                        rb.rearrange("p a b -> p (a b)"), channels=64)
                    for e in range(2):
                        for p in range(2):
                            j = 2 * p + e
                            # DVE: gpsimd TensorTensor requires equal start
                            # partitions across operands; DVE does not
                            nc.vector.tensor_mul(
                                outT_c[hg][qc][32 * j:32 * (j + 1), :],
                                a[32 * e:32 * e + 32, p, 256 * e:256 * (e + 1)],
                                bc[32 * e:32 * e + 32, p, 256 * e:256 * (e + 1)])
                    if hg == 1:
                        deferred_outproj.append(qc)
            emit_outproj(deferred_outproj.pop())

    nc.compile()
    _BUILD_CACHE["nc"] = nc
    return nc


def _run(x, w_qkv, b_qkv, w_out, trace=False):
    nc = build()
    x16 = np.asarray(x, np.float16)
    w16 = np.ascontiguousarray(np.asarray(w_qkv, np.float16))
    b16 = np.ascontiguousarray(np.asarray(b_qkv, np.float16).reshape(1, 3 * D))
    wo16 = np.ascontiguousarray(np.asarray(w_out, np.float16))
    in_maps = []
    for c in range(8):
        bi, qh = c // 2, c % 2
        in_maps.append({
            "xT": np.ascontiguousarray(x16[bi].T),
            "xqT": np.ascontiguousarray(x16[bi, NQ * qh:NQ * (qh + 1)].T),
            "w_qkv": w16,
            "b_qkv": b16,
            "w_out": wo16,
        })
    res = run_bass_kernel_spmd(nc, in_maps, core_ids=list(range(8)), trace=trace)
    out = np.empty((B, N, D), dtype=np.float32)
    for c in range(8):
        bi, qh = c // 2, c % 2
        out[bi, NQ * qh:NQ * (qh + 1)] = res.results[c]["out"]
    # v-bias correction (exact): attn@(v+b_v) = attn@v + b_v, so the device
    # omits b_v and the host adds its image through the output projection.
    bv = np.asarray(b_qkv, np.float32).reshape(-1)[2 * D:3 * D]
    out += (bv @ np.asarray(w_out, np.float32))[None, None, :]
    return out, res


def kernel(x, w_qkv, b_qkv, w_out, b_out):
    x = np.asarray(x, dtype=np.float32)
    out, _ = _run(x, np.asarray(w_qkv, np.float32), np.asarray(b_qkv, np.float32),
                  np.asarray(w_out, np.float32))
    return out + np.asarray(b_out, np.float32)[None, None, :]
